# revision 3
# baseline (speedup 1.0000x reference)
"""L21 norm kernel for Trainium2 (Bass/Tile), 8-core SPMD.

Computes sum_j sqrt(sum_i S[i,j]^2) for S of shape [8192, 16384] fp32.

Sharding: S is split along columns into 8 shards of [8192, 2048] (one per
NeuronCore). Each core computes the per-column L2 norms of its 2048
columns and DMAs the [1, 2048] norm vector out; the host sums all norms
in float64.

Per-core dataflow (memory-bound; 64 MiB HBM read per core; mid-stream
DMA measured at ~431 GB/s = the SBUF-AXI fabric ceiling, so all wins are
at the stream edges):
  - Bulk: 15 tiles of [128 partitions, 4 rows, 2048 cols] fp32 (4 MiB
    HWDGE DMAs; each partition's slice is 32 KiB contiguous in DRAM).
  - ACT engine: square with bf16 output (also the dtype cast for PE).
  - Partition-axis reduction is split so neither engine paces the DMA
    stream: per bulk tile, row-slices q=0,1 go to PE (ones[128,1]^T @ sq
    matmuls accumulating into PSUM [1,2048] fp32) and q=2,3 are
    accumulated on DVE into a bf16 [128,2048] accumulator (2x mode),
    folded into PSUM every 5 tiles (short bf16 chains for accuracy; PE
    has mid-stream slack).
  - Tail (rows 7680..8191): four [128, 1, 2048] slices with dedicated
    write-once buffers so their DMAs queue immediately behind the bulk
    stream (no buffer-reuse waits against ACT/PE progress). Slices 0-2
    are full-width and go straight to PE; slice 3 is DMA'd as two
    [128, 1024] column halves so the post-last-byte chain is
    square-half -> 2 matmuls -> sqrt-half, pipelined per half.
  - Epilogue: two ACT sqrts over [1, 1024] (each fires as soon as its
    PSUM banks get their stop-matmul), then one 8 KiB DMA of the
    [1, 2048] norms; the host sums them in float64.
"""

import numpy as np

# Full problem shape (hardcoded per the harness contract).
R = 8192          # rows
C_FULL = 16384    # columns
N_CORES = 8
C = C_FULL // N_CORES  # 2048 columns per core
P = 128           # SBUF partitions
NBLK = 512        # matmul moving free dim (one PSUM bank of fp32)
HALF = C // 2     # column half for the final tail slice

T4 = 15           # bulk tiles: [P, 4, C], rows 0..7680
ROWS4 = T4 * P * 4
# DVE-accumulator fold points (after the adds at tile t) and the tiles
# that restart the accumulator with a copy.
FOLD_TILES = (4, 9, T4 - 1)
RESET_TILES = (5, 10)

_cached = None


def _build():
    """Build + schedule the per-core Bass program. Returns the Bacc object."""
    import concourse.bacc as bacc
    import concourse.tile as tile
    from concourse import mybir

    nc = bacc.Bacc(
        "TRN2",
        target_bir_lowering=False,
        debug=False,
        enable_asserts=False,
        num_devices=N_CORES,
    )

    s_dram = nc.dram_tensor("S", [R, C], mybir.dt.float32, kind="ExternalInput")
    out_dram = nc.dram_tensor("out", [1, C], mybir.dt.float32, kind="ExternalOutput")

    s_ap = s_dram.ap()
    out_ap = out_dram.ap()

    # Bulk view [T4, P, 4, C]: partition p holds 4 consecutive rows ->
    # 32 KiB contiguous DRAM per (t, p) descriptor.
    v4 = s_ap[:ROWS4, :].rearrange("(t p q) c -> t p q c", p=P, q=4)
    # Tail: four [P, C] row-slices (1 MiB each).
    v1 = s_ap[ROWS4:, :].rearrange("(s p) c -> s p c", p=P)

    with tile.TileContext(nc) as tc:
        with (
            tc.tile_pool(name="io", bufs=3) as io_pool,
            tc.tile_pool(name="sqp", bufs=3) as sq_pool,
            tc.tile_pool(name="tio", bufs=4) as tio_pool,
            tc.tile_pool(name="tsq", bufs=4) as tsq_pool,
            tc.tile_pool(name="const", bufs=1) as const_pool,
            tc.tile_pool(name="ps", bufs=1, space="PSUM") as ps_pool,
            tc.tile_pool(name="fin", bufs=1) as fin_pool,
        ):
            # First input DMA before any const setup so streaming starts as
            # early as possible.
            x0 = io_pool.tile([P, 4, C], mybir.dt.float32, tag="x")
            # Issued from the ACT engine's HWDGE ring: its preamble clears
            # earlier than Sync's, so streaming starts sooner.
            nc.scalar.dma_start(out=x0, in_=v4[0])

            ones = const_pool.tile([P, 1], mybir.dt.bfloat16)
            nc.vector.memset(ones, 1.0)

            # DVE-side accumulator for q=2,3 row-slices.
            acc = const_pool.tile([P, C], mybir.dt.bfloat16)

            # Per-column sum of squares (4 PSUM banks).
            colsq = ps_pool.tile([1, C], mybir.dt.float32)

            # Dummy sqrt: pulls the sqrt ACT-table load out of the tail.
            warm = const_pool.tile([1, 1], mybir.dt.float32)
            nc.scalar.sqrt(out=warm, in_=ones[0:1, :])

            def pe_reduce(src, first=False, blocks=range(C // NBLK), stop_blocks=()):
                for b in blocks:
                    nc.tensor.matmul(
                        colsq[:, b * NBLK : (b + 1) * NBLK],
                        ones,
                        src[:, b * NBLK : (b + 1) * NBLK],
                        start=first,
                        stop=(b in stop_blocks),
                    )

            for t in range(T4):
                if t == 0:
                    x_tile = x0
                else:
                    x_tile = io_pool.tile([P, 4, C], mybir.dt.float32, tag="x")
                    nc.sync.dma_start(out=x_tile, in_=v4[t])

                sq = sq_pool.tile([P, 4, C], mybir.dt.bfloat16, tag="sq")
                nc.scalar.square(out=sq, in_=x_tile)

                pe_reduce(sq[:, 0, :], first=(t == 0))
                pe_reduce(sq[:, 1, :])

                if t == 0 or t in RESET_TILES:
                    nc.vector.tensor_copy(acc, sq[:, 2, :])
                else:
                    nc.vector.tensor_add(acc, acc, sq[:, 2, :])
                nc.vector.tensor_add(acc, acc, sq[:, 3, :])

                if t in FOLD_TILES:
                    pe_reduce(acc)

            # Tail row-slices: dedicated write-once buffers -> their DMAs
            # queue immediately behind bulk tile 14 on the sync HWDGE ring
            # with no buffer-reuse waits. Slices 0-2 full width; slice 3 as
            # two column halves to shorten the post-last-byte chain.
            xt = [
                tio_pool.tile([P, 1, C], mybir.dt.float32, tag="xt", name=f"xt{s}")
                for s in range(4)
            ]
            sqt = [
                tsq_pool.tile([P, 1, C], mybir.dt.bfloat16, tag="sqt", name=f"sqt{s}")
                for s in range(4)
            ]
            for s in range(3):
                nc.sync.dma_start(out=xt[s][:, 0, :], in_=v1[s])
            nc.sync.dma_start(out=xt[3][:, 0, :HALF], in_=v1[3][:, :HALF])
            nc.sync.dma_start(out=xt[3][:, 0, HALF:], in_=v1[3][:, HALF:])

            for s in range(3):
                nc.scalar.square(out=sqt[s], in_=xt[s])
                pe_reduce(sqt[s][:, 0, :])

            norms = fin_pool.tile([1, C], mybir.dt.float32)

            # Half A: square cols 0..HALF, final (stop) matmuls for blocks
            # 0..1, then sqrt those banks while half B is still in flight.
            nc.scalar.square(out=sqt[3][:, :, :HALF], in_=xt[3][:, :, :HALF])
            pe_reduce(sqt[3][:, 0, :], blocks=range(HALF // NBLK),
                      stop_blocks=set(range(HALF // NBLK)))
            nc.scalar.activation(
                norms[:, :HALF], colsq[:, :HALF], mybir.ActivationFunctionType.Sqrt
            )

            # Half B: last bytes of the stream -> shortest possible chain.
            nc.scalar.square(out=sqt[3][:, :, HALF:], in_=xt[3][:, :, HALF:])
            pe_reduce(sqt[3][:, 0, :], blocks=range(HALF // NBLK, C // NBLK),
                      stop_blocks=set(range(HALF // NBLK, C // NBLK)))
            nc.scalar.activation(
                norms[:, HALF:], colsq[:, HALF:], mybir.ActivationFunctionType.Sqrt
            )

            nc.sync.dma_start(out=out_ap, in_=norms)

    nc.compile()
    return nc


def _get_nc():
    global _cached
    if _cached is None:
        _cached = _build()
    return _cached


def _run(S: np.ndarray, trace: bool = False):
    from concourse import bass_utils

    assert S.shape == (R, C_FULL), S.shape
    S = np.ascontiguousarray(np.asarray(S, dtype=np.float32))

    nc = _get_nc()
    in_maps = [
        {"S": np.ascontiguousarray(S[:, i * C : (i + 1) * C])} for i in range(N_CORES)
    ]
    try:
        res = bass_utils.run_bass_kernel_spmd(
            nc, in_maps, core_ids=list(range(N_CORES)), trace=trace
        )
    except Exception:
        # One retry: transient NRT/device hiccups (e.g. a wedged core from a
        # previous process) are recoverable on re-execution.
        res = bass_utils.run_bass_kernel_spmd(
            nc, in_maps, core_ids=list(range(N_CORES)), trace=trace
        )
    partials = np.array(
        [np.asarray(res.results[i]["out"], dtype=np.float64).sum() for i in range(N_CORES)],
        dtype=np.float64,
    )
    out = np.float32(partials.sum())
    return out, res


def kernel(S: np.ndarray) -> np.ndarray:
    out, _ = _run(S, trace=False)
    return np.asarray(out, dtype=np.float32)


def run_traced(S: np.ndarray):
    """For test.py: returns (output, BassKernelResults) with NTFF trace."""
    return _run(S, trace=True)


# revision 7
# speedup vs baseline: 1.2142x; 1.2142x over previous
"""L21 norm kernel for Trainium2 (Bass/Tile), 8-core SPMD.

Computes sum_j sqrt(sum_i S[i,j]^2) for S of shape [8192, 16384] fp32.

Sharding: S is split along columns into 8 shards of [8192, 2048] (one per
NeuronCore). Each core computes the per-column L2 norms of its 2048
columns and DMAs the [1, 2048] norm vector out; the host sums all norms
in float64.

Per-core dataflow (memory-bound; 64 MiB HBM read per core; mid-stream
DMA measured at ~431 GB/s = the SBUF-AXI fabric ceiling, so all wins are
at the stream edges):
  - Bulk: 15 tiles of [128 partitions, 4 rows, 2048 cols] fp32 (4 MiB
    HWDGE DMAs; each partition's slice is 32 KiB contiguous in DRAM).
  - ACT engine: square with bf16 output (also the dtype cast for PE).
  - Partition-axis reduction is split so neither engine paces the DMA
    stream: per bulk tile, row-slices q=0,1 go to PE (ones[128,1]^T @ sq
    matmuls accumulating into PSUM [1,2048] fp32) and q=2,3 are
    accumulated on DVE into a bf16 [128,2048] accumulator (2x mode),
    folded into PSUM every 5 tiles (short bf16 chains for accuracy; PE
    has mid-stream slack).
  - Tail (rows 7680..8191): four [128, 1, 2048] slices with dedicated
    write-once buffers so their DMAs queue immediately behind the bulk
    stream (no buffer-reuse waits against ACT/PE progress). Slices 0-2
    are full-width and are absorbed by the DVE accumulator; slice 3 is
    DMA'd as four [128, 512] column quarters. The accumulator folds into
    PSUM per column block (block b gated only on quarter b), so just one
    matmul and one [1, 512] sqrt chunk trail the last byte.
  - Epilogue: four chunked ACT sqrts (each fires as soon as its PSUM
    bank gets its stop-matmul), then one 8 KiB DMA of the [1, 2048]
    norms; the host sums them in float64.
"""

import numpy as np

# Full problem shape (hardcoded per the harness contract).
R = 8192          # rows
C_FULL = 16384    # columns
N_CORES = 8
C = C_FULL // N_CORES  # 2048 columns per core
P = 128           # SBUF partitions
NBLK = 512        # matmul moving free dim (one PSUM bank of fp32)

T4 = 15           # bulk tiles: [P, 4, C], rows 0..7680
ROWS4 = T4 * P * 4
# DVE-accumulator fold points (after the adds at tile t) and the tiles
# that restart the accumulator with a copy. The last stretch (tiles 10-14
# plus the tail row-slices) folds once, per column block, in the epilogue.
FOLD_TILES = (4, 9)
RESET_TILES = (5, 10)
NQ = 4            # column quarters of the final tail slice
QW = C // NQ      # 512 columns per quarter

_cached = None


def _build():
    """Build + schedule the per-core Bass program. Returns the Bacc object."""
    import concourse.bacc as bacc
    import concourse.tile as tile
    from concourse import mybir

    nc = bacc.Bacc(
        "TRN2",
        target_bir_lowering=False,
        debug=False,
        enable_asserts=False,
        num_devices=N_CORES,
    )

    s_dram = nc.dram_tensor("S", [R, C], mybir.dt.float32, kind="ExternalInput")
    out_dram = nc.dram_tensor("out", [1, C], mybir.dt.float32, kind="ExternalOutput")

    s_ap = s_dram.ap()
    out_ap = out_dram.ap()

    # Bulk view [T4, P, 4, C]: partition p holds 4 consecutive rows ->
    # 32 KiB contiguous DRAM per (t, p) descriptor.
    v4 = s_ap[:ROWS4, :].rearrange("(t p q) c -> t p q c", p=P, q=4)
    # Tail: four [P, C] row-slices (1 MiB each).
    v1 = s_ap[ROWS4:, :].rearrange("(s p) c -> s p c", p=P)

    with tile.TileContext(nc) as tc:
        with (
            tc.tile_pool(name="io", bufs=3) as io_pool,
            tc.tile_pool(name="sqp", bufs=3) as sq_pool,
            tc.tile_pool(name="tio", bufs=4) as tio_pool,
            tc.tile_pool(name="tsq", bufs=4) as tsq_pool,
            tc.tile_pool(name="const", bufs=1) as const_pool,
            tc.tile_pool(name="ps", bufs=1, space="PSUM") as ps_pool,
            tc.tile_pool(name="fin", bufs=1) as fin_pool,
        ):
            # First input DMA before any const setup so streaming starts as
            # early as possible.
            x0 = io_pool.tile([P, 4, C], mybir.dt.float32, tag="x")
            # Issued from the ACT engine's HWDGE ring: its preamble clears
            # earlier than Sync's, so streaming starts sooner.
            nc.scalar.dma_start(out=x0, in_=v4[0])

            ones = const_pool.tile([P, 1], mybir.dt.bfloat16)
            nc.vector.memset(ones, 1.0)

            # DVE-side accumulator for q=2,3 row-slices.
            acc = const_pool.tile([P, C], mybir.dt.bfloat16)

            # Per-column sum of squares (4 PSUM banks).
            colsq = ps_pool.tile([1, C], mybir.dt.float32)

            # Dummy sqrt: pulls the sqrt ACT-table load out of the tail.
            warm = const_pool.tile([1, 1], mybir.dt.float32)
            nc.scalar.sqrt(out=warm, in_=ones[0:1, :])

            def pe_reduce(src, first=False, blocks=range(C // NBLK), stop_blocks=()):
                for b in blocks:
                    nc.tensor.matmul(
                        colsq[:, b * NBLK : (b + 1) * NBLK],
                        ones,
                        src[:, b * NBLK : (b + 1) * NBLK],
                        start=first,
                        stop=(b in stop_blocks),
                    )

            for t in range(T4):
                if t == 0:
                    x_tile = x0
                else:
                    x_tile = io_pool.tile([P, 4, C], mybir.dt.float32, tag="x")
                    nc.sync.dma_start(out=x_tile, in_=v4[t])

                sq = sq_pool.tile([P, 4, C], mybir.dt.bfloat16, tag="sq")
                nc.scalar.square(out=sq, in_=x_tile)

                pe_reduce(sq[:, 0, :], first=(t == 0))
                pe_reduce(sq[:, 1, :])

                if t == 0 or t in RESET_TILES:
                    nc.vector.tensor_copy(acc, sq[:, 2, :])
                else:
                    nc.vector.tensor_add(acc, acc, sq[:, 2, :])
                nc.vector.tensor_add(acc, acc, sq[:, 3, :])

                if t in FOLD_TILES:
                    pe_reduce(acc)

            # Tail row-slices: dedicated write-once buffers -> their DMAs
            # queue immediately behind bulk tile 14 on the sync HWDGE ring
            # with no buffer-reuse waits. Slices 0-2 are full width and are
            # absorbed by the DVE accumulator (no PE work at the tail until
            # the fold); slice 3 is DMA'd as four column quarters so the
            # post-last-byte chain per quarter is square -> add -> fold
            # matmul -> sqrt, with only the last quarter on the critical
            # path.
            xt = [
                tio_pool.tile([P, 1, C], mybir.dt.float32, tag="xt", name=f"xt{s}")
                for s in range(4)
            ]
            sqt = [
                tsq_pool.tile([P, 1, C], mybir.dt.bfloat16, tag="sqt", name=f"sqt{s}")
                for s in range(4)
            ]
            for s in range(3):
                nc.sync.dma_start(out=xt[s][:, 0, :], in_=v1[s])
            for q in range(NQ):
                nc.sync.dma_start(
                    out=xt[3][:, 0, q * QW : (q + 1) * QW],
                    in_=v1[3][:, q * QW : (q + 1) * QW],
                )

            # ACT squares (strict FIFO: each fires as its DMA lands) and
            # DVE adds into the accumulator.
            for s in range(3):
                nc.scalar.square(out=sqt[s], in_=xt[s])
                nc.vector.tensor_add(acc, acc, sqt[s][:, 0, :])
            for q in range(NQ):
                cols = slice(q * QW, (q + 1) * QW)
                nc.scalar.square(out=sqt[3][:, :, cols], in_=xt[3][:, :, cols])
                nc.vector.tensor_add(
                    acc[:, cols], acc[:, cols], sqt[3][:, 0, cols]
                )

            # Per-block fold (the only tail matmuls) with stop flags; block
            # b only needs quarter b's add, so blocks 0-2 fold while the
            # last quarter is still in flight.
            for b in range(C // NBLK):
                pe_reduce(acc, blocks=(b,), stop_blocks=(b,))

            # Chunked sqrt: chunk b fires once block b's fold lands.
            norms = fin_pool.tile([1, C], mybir.dt.float32)
            for b in range(C // NBLK):
                cols = slice(b * NBLK, (b + 1) * NBLK)
                nc.scalar.activation(
                    norms[:, cols], colsq[:, cols],
                    mybir.ActivationFunctionType.Sqrt,
                )

            nc.sync.dma_start(out=out_ap, in_=norms)

    nc.compile()
    return nc


def _get_nc():
    global _cached
    if _cached is None:
        _cached = _build()
    return _cached


def _run(S: np.ndarray, trace: bool = False):
    from concourse import bass_utils

    assert S.shape == (R, C_FULL), S.shape
    S = np.ascontiguousarray(np.asarray(S, dtype=np.float32))

    nc = _get_nc()
    in_maps = [
        {"S": np.ascontiguousarray(S[:, i * C : (i + 1) * C])} for i in range(N_CORES)
    ]
    try:
        res = bass_utils.run_bass_kernel_spmd(
            nc, in_maps, core_ids=list(range(N_CORES)), trace=trace
        )
    except Exception:
        # One retry: transient NRT/device hiccups (e.g. a wedged core from a
        # previous process) are recoverable on re-execution.
        res = bass_utils.run_bass_kernel_spmd(
            nc, in_maps, core_ids=list(range(N_CORES)), trace=trace
        )
    partials = np.array(
        [np.asarray(res.results[i]["out"], dtype=np.float64).sum() for i in range(N_CORES)],
        dtype=np.float64,
    )
    out = np.float32(partials.sum())
    return out, res


def kernel(S: np.ndarray) -> np.ndarray:
    out, _ = _run(S, trace=False)
    return np.asarray(out, dtype=np.float32)


def run_traced(S: np.ndarray):
    """For test.py: returns (output, BassKernelResults) with NTFF trace."""
    return _run(S, trace=True)
